# revision 14
# baseline (speedup 1.0000x reference)
"""Trainium2 Bass kernel for the ConcreteLayer training forward pass.

Computes out = x @ softmax((weight - ln(-ln((1-tiny)*uniform + tiny))) / T, axis=1)

Strategy (8 NeuronCores, pure data-parallel, zero collectives):
  - x sharded along batch 8 ways (512 rows/core, shipped pre-transposed in
    bf16); weight/uniform/T replicated.  Each core computes the FULL
    softmax (all 4096x1024 logits) redundantly -- the row-sum over the out
    dim then never crosses cores, which removes the ncfw collective
    latency (~19-49us per 2-rank AllGather + ~67us init barrier) that
    dominated the exchange-based variant.
  - The elementwise chain stays SHORT and ACT-centric (spreading it over
    DVE+GpSimd measured slower: every op lost 20-25% to SBUF contention
    and the 5-hop chain pushed the first matmul to t=60us): per 4-ktile
    chunk, DMA u,w -> Ln, Ln (ACT) -> sub (DVE) -> exp with row-sum
    accumulation (ACT) -> reciprocal + normalize (DVE), samples resident
    in bf16.  Triple-buffered chunk tiles keep the rings and ACT fed
    (2 buffers measured every engine at ~70% on dependency stalls).
  - DMA: u/w stream on the SP HWDGE ring; xt/out ride the ACT HWDGE ring
    (split rings measured 350-410 GB/s vs ~240 single-ring).
  - GEMM: out[512, 1024] accumulated over 32 contraction tiles into 4
    PSUM tiles of [128, 1024] (all 8 banks), bf16 operands, N=512 per
    matmul; each ktile's 8 matmuls issue right after its normalize so PE
    gaps stay small and the kernel tail is one ktile deep.
  - Host only transposes/casts/slices x and concatenates the 8 output
    shards.
"""

import sys

import numpy as np

for _p in ("/opt/trn_rl_repo",):
    if _p not in sys.path:
        sys.path.insert(0, _p)

B, IN, OUT = 4096, 4096, 1024
P = 128
KT = IN // P  # 32 contraction tiles
KG = 4  # ktiles per softmax chunk
NCH = KT // KG  # 8 softmax chunks
NCORES = 8
BS = B // NCORES  # 512 batch rows per core
MBT = BS // P  # 4 output row tiles per core
NH = OUT // 512  # 2 matmul col halves
XKG = 4  # ktiles per xt DMA chunk (512 KB)
TINY = float(np.finfo(np.float32).tiny)

_PROGRAM = None
LAST_RESULT = None


def _pin_act_tables():
    """Steer the act-table-load pass to one set (has both Ln and Exp) so the
    compiler emits one ACT_TABLE_LOAD instead of reloading per chunk."""
    import concourse.mybir as mybir
    from concourse import bacc, hw_specs

    orig = hw_specs.get_activation_tables.__wrapped__
    target = "natural_log_exp_and_others"
    strip = {
        mybir.ActivationFunctionType.Ln,
        mybir.ActivationFunctionType.Exp,
    }

    def pinned(arch):
        tables = orig(arch)
        if target not in tables:
            return tables
        return {
            name: (set(fns) if name == target else {f for f in fns if f not in strip})
            for name, fns in tables.items()
        }

    bacc.get_activation_tables = pinned


def _build_program():
    import concourse.bass as bass
    import concourse.mybir as mybir
    import concourse.tile as tile
    from concourse import bacc
    from contextlib import ExitStack

    _pin_act_tables()

    f32 = mybir.dt.float32
    bf16 = mybir.dt.bfloat16
    Ln = mybir.ActivationFunctionType.Ln
    Exp = mybir.ActivationFunctionType.Exp

    nc = bacc.Bacc(
        "TRN2", target_bir_lowering=False, debug=False, num_devices=NCORES
    )

    xt_d = nc.dram_tensor("xt", [IN, BS], bf16, kind="ExternalInput")
    w_d = nc.dram_tensor("wf", [IN, OUT], f32, kind="ExternalInput")
    u_d = nc.dram_tensor("uf", [IN, OUT], f32, kind="ExternalInput")
    t_d = nc.dram_tensor("tt", [1], f32, kind="ExternalInput")
    out_d = nc.dram_tensor("out", [BS, OUT], f32, kind="ExternalOutput")

    with tile.TileContext(nc) as tc, ExitStack() as ctx:
        singles = ctx.enter_context(tc.tile_pool(name="singles", bufs=1))
        chunks = ctx.enter_context(tc.tile_pool(name="chunks", bufs=3))
        outp = ctx.enter_context(tc.tile_pool(name="outp", bufs=2))
        psum = ctx.enter_context(tc.tile_pool(name="psum", bufs=1, space="PSUM"))

        # 1/T broadcast to all partitions.
        t_sb = singles.tile([P, 1], f32)
        t_ap = t_d.ap()
        nc.sync.dma_start(
            out=t_sb, in_=bass.AP(tensor=t_ap.tensor, offset=0, ap=[[0, P], [1, 1]])
        )
        invt = singles.tile([P, 1], f32)
        nc.vector.reciprocal(invt, t_sb)

        zero_t = singles.tile([P, 1], f32)
        nc.vector.memset(zero_t, 0.0)
        tiny_t = singles.tile([P, 1], f32)
        nc.vector.memset(tiny_t, TINY)

        # Normalized softmax samples, resident, bf16 for full-rate GEMM.
        e_all = singles.tile([P, KT, OUT], bf16)
        sums = singles.tile([P, KT], f32)
        invd = singles.tile([P, KT], f32)
        # x slice (transposed, bf16), resident.
        xt_all = singles.tile([P, KT, BS], bf16)

        ps_tiles = [
            psum.tile([P, OUT], f32, tag=f"ps{mb}", name=f"ps{mb}")
            for mb in range(MBT)
        ]

        def gemm_ktile(ki):
            for mb in range(MBT):
                for nh in range(NH):
                    nc.tensor.matmul(
                        ps_tiles[mb][:, nh * 512 : (nh + 1) * 512],
                        lhsT=xt_all[:, ki, mb * P : (mb + 1) * P],
                        rhs=e_all[:, ki, nh * 512 : (nh + 1) * 512],
                        start=(ki == 0),
                        stop=(ki == KT - 1),
                    )

        def norm_gemm_ktile(ki):
            # samples = e * (1/rowsum), then this ktile's 8 matmuls.
            nc.vector.tensor_scalar_mul(
                e_all[:, ki, :], e_all[:, ki, :], invd[:, ki : ki + 1]
            )
            gemm_ktile(ki)

        def softmax_chunk(ch):
            """Emit chunk ch's logits pipeline, with chunk ch-1's
            normalize+GEMM interleaved between the per-ktile subs.  Engine
            queues are strict FIFO, so this emission order spaces the PE's
            8-matmul bursts ~1.5-2us apart instead of releasing a 32-matmul
            burst every ~13us with a >3.4us idle gap in between -- which
            re-throttled the PE clock (HAM) every chunk (MATMUL busy
            measured 109us vs the ~60us warm cost)."""
            base = ch * KG * P
            u_t = chunks.tile([P, KG, OUT], f32, tag="u", name="u_t")
            w_t = chunks.tile([P, KG, OUT], f32, tag="w", name="w_t")
            u_src = u_d[base : base + KG * P, :].rearrange("(g p) c -> p g c", p=P)
            w_src = w_d[base : base + KG * P, :].rearrange("(g p) c -> p g c", p=P)
            nc.sync.dma_start(out=u_t, in_=u_src)
            nc.sync.dma_start(out=w_t, in_=w_src)
            # v = ln((1 - tiny)*u + tiny)            (negative)
            nc.scalar.activation(u_t, u_t, Ln, bias=tiny_t[:], scale=1.0 - TINY)
            # m = ln(-v) = -gumbel
            nc.scalar.activation(u_t, u_t, Ln, bias=zero_t[:], scale=-1.0)
            for g in range(KG):
                # d = w - m = w + gumbel  (per ktile, so the DVE stream
                # alternates sub(ch) / norm(ch-1) and paces the PE)
                nc.vector.tensor_sub(
                    u_t[:, g, :], w_t[:, g, :], u_t[:, g, :]
                )
                if ch > 0:
                    norm_gemm_ktile((ch - 1) * KG + g)
            # e = exp(d / T); accumulate row-sums on the ACT accumulator.
            for g in range(KG):
                ki = ch * KG + g
                nc.scalar.activation(
                    e_all[:, ki, :],
                    u_t[:, g, :],
                    Exp,
                    bias=zero_t[:],
                    scale=invt[:],
                    accum_out=sums[:, ki : ki + 1],
                )
            sl = slice(ch * KG, (ch + 1) * KG)
            nc.vector.reciprocal(invd[:, sl], sums[:, sl])

        def xt_chunk(xc):
            base = xc * XKG * P
            xt_src = xt_d[base : base + XKG * P, :].rearrange("(g p) b -> p g b", p=P)
            # ACT-ring HWDGE: keeps xt off the u/w SP ring.  (Routing these
            # through GpSimd SWDGE instead measured 165.2us vs 161.0us.)
            nc.scalar.dma_start(
                out=xt_all[:, xc * XKG : (xc + 1) * XKG, :], in_=xt_src
            )

        for ch in range(NCH):
            softmax_chunk(ch)
            # Emit xt AFTER the chunk: its ~2.5us dma_start issue then can't
            # sit ahead of the first Ln on the ACT queue; the ktiles it feeds
            # aren't consumed until the next chunk's interleaved GEMM.
            if ch < KT // XKG:
                xt_chunk(ch)
        # Drain the software pipeline: last chunk's normalize + GEMM.
        for g in range(KG):
            norm_gemm_ktile((NCH - 1) * KG + g)

        for mb in range(MBT):
            o_t = outp.tile([P, OUT], f32, tag="o")
            nc.vector.tensor_copy(o_t, ps_tiles[mb][:])
            nc.scalar.dma_start(out=out_d[mb * P : (mb + 1) * P, :], in_=o_t)

    nc.compile()
    return nc


def kernel(x, weight, uniform, T):
    global _PROGRAM, LAST_RESULT
    import ml_dtypes
    from concourse.bass_utils import run_bass_kernel_spmd

    if _PROGRAM is None:
        _PROGRAM = _build_program()
    nc = _PROGRAM

    x = np.asarray(x, dtype=np.float32)
    weight = np.ascontiguousarray(np.asarray(weight, dtype=np.float32))
    uniform = np.ascontiguousarray(np.asarray(uniform, dtype=np.float32))
    T = np.ascontiguousarray(np.asarray(T, dtype=np.float32)).reshape([1])

    xt16 = np.ascontiguousarray(x.T).astype(ml_dtypes.bfloat16)  # [IN, B]
    in_maps = []
    for c in range(NCORES):
        in_maps.append(
            {
                "xt": np.ascontiguousarray(xt16[:, c * BS : (c + 1) * BS]),
                "wf": weight,
                "uf": uniform,
                "tt": T,
            }
        )

    res = run_bass_kernel_spmd(nc, in_maps, core_ids=list(range(NCORES)))
    LAST_RESULT = res

    out = np.empty((B, OUT), dtype=np.float32)
    for c in range(NCORES):
        out[c * BS : (c + 1) * BS, :] = res.results[c]["out"]
    return out
